# revision 18
# baseline (speedup 1.0000x reference)
"""nn_Decoder kernel for 8 TRN2 NeuronCores.

Model: masked LSTM decoder (Keras semantics) + Luong dot attention.
  mask = dec_inputs != 0
  x = E[dec_inputs]                       [B,T,D]
  dec_outputs, hT, cT = masked_lstm(x, mask, h0, c0, W, U, b)
  scores = dec_outputs @ enc^T ; probs = softmax(scores)
  ctx = probs @ enc ; ctx[~mask] = 0
  returns (ctx, hT, cT)

Sharding: data-parallel over batch, 8 rows per core. The LSTM recurrence
runs in transposed layout (hidden dim on partitions, batch on free dim)
with U as the stationary matmul operand (bf16, FWL weight loads).
x@W is precomputed for all timesteps in one batched matmul (bf16) and
staged through DRAM in 64-step blocks. Attention runs in float32r.

Numerics (validated by emulation + a HW probe): bf16 recurrence matmuls
with fp32 cell/hidden carry give ~2.3e-3 absmax error vs fp32 reference;
float32r matmul measures 1.5e-4 relative error on HW.
"""
import numpy as np
import ml_dtypes
from contextlib import ExitStack

import concourse.bass as bass
import concourse.bacc as bacc
import concourse.tile as tile
from concourse import mybir
from concourse.bass_utils import run_bass_kernel_spmd
from concourse.masks import make_identity

AF = mybir.ActivationFunctionType
ALU = mybir.AluOpType
DT = mybir.dt
F32 = DT.float32
F32R = DT.float32r
BF16 = DT.bfloat16
I32 = DT.int32

B, TE, H, D, V = 64, 256, 512, 256, 50000
FH = 4 * H            # 2048
NCORES = 8
BL = B // NCORES      # 8 batch rows per core
HK = H // 128         # 4 contraction k-tiles over H
MT = FH // 128        # 16 m-tiles over 4H
DK = D // 128         # 2 k-tiles over D
KT = TE // 128        # 2 key tiles for attention
XWBLK = 32            # timesteps per XW^T DRAM->SBUF block


def build_kernel(T=256):
    NF = T * BL                      # flats = (t, b) pairs, t-major
    n_chunks = NF // 128             # gather chunks of 128 rows
    QT = T // 128                    # query tiles for attention

    nc = bacc.Bacc("TRN2", target_bir_lowering=False, debug=False,
                   num_devices=NCORES)

    E_d = nc.dram_tensor("E", [V, D], BF16, kind="ExternalInput").ap()
    W_d = nc.dram_tensor("W", [D, FH], BF16, kind="ExternalInput").ap()
    U_d = nc.dram_tensor("U", [H, FH], BF16, kind="ExternalInput").ap()
    bT_d = nc.dram_tensor("bT", [128, MT], F32, kind="ExternalInput").ap()
    decT_d = nc.dram_tensor("decT", [T, BL], I32, kind="ExternalInput").ap()
    decT4_d = nc.dram_tensor("decT4", [T, HK, BL], I32, kind="ExternalInput").ap()
    idxw_d = nc.dram_tensor("idxw", [128, n_chunks], I32, kind="ExternalInput").ap()
    h0_d = nc.dram_tensor("h0T", [128, HK, BL], F32, kind="ExternalInput").ap()
    c0_d = nc.dram_tensor("c0T", [128, HK, BL], F32, kind="ExternalInput").ap()
    encT_d = nc.dram_tensor("encT", [BL, H, TE], F32, kind="ExternalInput").ap()
    enc_d = nc.dram_tensor("enc", [BL, TE, H], F32, kind="ExternalInput").ap()

    ctx_d = nc.dram_tensor("ctx8", [BL, QT, 128, H], F32, kind="ExternalOutput").ap()
    hc_d = nc.dram_tensor("hc", [2, 128, HK, BL], F32, kind="ExternalOutput").ap()

    with tile.TileContext(nc) as tc:
        with ExitStack() as ctx:
            sing = ctx.enter_context(tc.tile_pool(name="sing", bufs=1))
            ph1 = ctx.enter_context(tc.tile_pool(name="ph1", bufs=1))
            xpool = ctx.enter_context(tc.tile_pool(name="xpool", bufs=3))
            xwst = ctx.enter_context(tc.tile_pool(name="xwst", bufs=3))
            xwbp = ctx.enter_context(tc.tile_pool(name="xwbp", bufs=2))
            loop = ctx.enter_context(tc.tile_pool(name="loop", bufs=3))
            att = ctx.enter_context(tc.tile_pool(name="att", bufs=2))
            attr = ctx.enter_context(tc.tile_pool(name="attr", bufs=1))
            attc = ctx.enter_context(tc.tile_pool(name="attc", bufs=2))
            mseg = ctx.enter_context(tc.tile_pool(name="mseg", bufs=2))
            dram = ctx.enter_context(tc.tile_pool(name="dram", bufs=1, space="DRAM"))
            psA = ctx.enter_context(tc.tile_pool(name="psA", bufs=3, space="PSUM"))
            psZ = ctx.enter_context(tc.tile_pool(name="psZ", bufs=1, space="PSUM"))

            # ---------- setup ----------
            ident_bf = sing.tile([128, 128], BF16)
            make_identity(nc, ident_bf)
            ident_f = sing.tile([128, 128], F32)
            make_identity(nc, ident_f)
            ident_r = sing.tile([128, 128], F32R)
            nc.vector.tensor_copy(ident_r, ident_f)

            U_bf = sing.tile([128, HK, FH], BF16)
            nc.sync.dma_start(out=U_bf, in_=U_d.rearrange("(k p) f -> p k f", p=128))
            W_bf = ph1.tile([128, DK, FH], BF16)
            nc.sync.dma_start(out=W_bf, in_=W_d.rearrange("(k p) f -> p k f", p=128))
            bT_sb = sing.tile([128, MT], F32)
            nc.sync.dma_start(out=bT_sb, in_=bT_d)
            idx_sb = sing.tile([128, n_chunks], I32)
            nc.sync.dma_start(out=idx_sb, in_=idxw_d)
            h_run = sing.tile([128, HK, BL], F32)
            nc.sync.dma_start(out=h_run, in_=h0_d)
            c_run = sing.tile([128, HK, BL], F32)
            nc.sync.dma_start(out=c_run, in_=c0_d)
            h_bf = sing.tile([128, HK, BL], BF16)
            nc.scalar.copy(h_bf, h_run)

            # masks
            decq_sb = sing.tile([128, QT, BL], I32)
            nc.sync.dma_start(out=decq_sb,
                              in_=decT_d.rearrange("(q p) b -> p q b", p=128))
            maskTq = sing.tile([128, QT, BL], F32)
            nc.vector.tensor_scalar(out=maskTq, in0=decq_sb, scalar1=0,
                                    scalar2=None, op0=ALU.not_equal)
            # mask broadcast to all 128 partitions via stride-0 DMA reads
            maskbc = sing.tile([128, NF * HK], DT.uint8)
            dec4_flat = decT4_d.rearrange("t k b -> (t k b)")
            seg = NF * HK // 4
            for s in range(4):
                mstg = mseg.tile([128, seg], I32, name=f"mstg_{s}", tag="mstg")
                src = bass.AP(tensor=dec4_flat.tensor,
                              offset=dec4_flat.offset + s * seg,
                              ap=[[0, 128], [1, seg]])
                nc.sync.dma_start(out=mstg, in_=src)
                nc.vector.tensor_scalar(out=maskbc[:, s * seg:(s + 1) * seg],
                                        in0=mstg, scalar1=0,
                                        scalar2=None, op0=ALU.not_equal)

            hsT = sing.tile([128, HK, T, BL], F32R)
            xw_dram = dram.tile([128, MT, NF], BF16)

            # ---------- phase 1: embedding gather + XW^T ----------
            xT = ph1.tile([128, DK, NF], BF16)
            for r in range(n_chunks):
                x_sb = xpool.tile([128, D], BF16, name=f"x_{r}", tag="x")
                nc.gpsimd.indirect_dma_start(
                    out=x_sb, out_offset=None, in_=E_d,
                    in_offset=bass.IndirectOffsetOnAxis(ap=idx_sb[:, r:r + 1], axis=0))
                for k in range(DK):
                    pt = psA.tile([128, 128], BF16, name=f"pt_{r}_{k}", tag="mmA")
                    nc.tensor.transpose(out=pt, in_=x_sb[:, k * 128:(k + 1) * 128],
                                        identity=ident_bf)
                    nc.vector.tensor_copy(xT[:, k, r * 128:(r + 1) * 128], pt)

            n_nc = NF // 512
            for m in range(MT):
                for c in range(n_nc):
                    pxw = psA.tile([128, 512], F32, name=f"pxw_{m}_{c}", tag="mmA")
                    for k in range(DK):
                        nc.tensor.matmul(out=pxw,
                                         lhsT=W_bf[:, k, m * 128:(m + 1) * 128],
                                         rhs=xT[:, k, c * 512:(c + 1) * 512],
                                         start=(k == 0), stop=(k == DK - 1))
                    stg = xwst.tile([128, 512], BF16, name=f"stg_{m}_{c}", tag="stg")
                    if (m + c) % 2 == 0:
                        nc.scalar.activation(out=stg, in_=pxw, func=AF.Identity,
                                             bias=bT_sb[:, m:m + 1], scale=1.0)
                    else:
                        nc.vector.tensor_scalar(out=stg, in0=pxw,
                                                scalar1=bT_sb[:, m:m + 1],
                                                scalar2=None, op0=ALU.add)
                    nc.sync.dma_start(out=xw_dram[:, m, c * 512:(c + 1) * 512],
                                      in_=stg)

            # ---------- phase 2: recurrence ----------
            xwblk = None
            for t in range(T):
                blk, toff = divmod(t, XWBLK)
                if toff == 0:
                    nb = min(XWBLK, T - blk * XWBLK)
                    xwblk = xwbp.tile([128, MT, XWBLK * BL], BF16,
                                      name=f"xwblk_{blk}", tag="xwblk")
                    nc.sync.dma_start(
                        out=xwblk[:, :, :nb * BL],
                        in_=xw_dram[:, :, blk * XWBLK * BL:(blk * XWBLK + nb) * BL])

                KB = HK * BL
                m3 = maskbc[:, t * KB:(t + 1) * KB]
                s_i = s_f = t_g = m1 = m2 = tan_c = None
                for g in range(4):
                    zp = psZ.tile([128, HK, BL], F32, name=f"zp{g}_{t}", tag=f"z{g}")
                    for j in range(HK):
                        mtile = g * 4 + j
                        for k in range(HK):
                            nc.tensor.matmul(
                                out=zp[:, j, :],
                                lhsT=U_bf[:, k, mtile * 128:(mtile + 1) * 128],
                                rhs=h_bf[:, k, :],
                                start=(k == 0), stop=(k == HK - 1))
                    z = loop.tile([128, HK, BL], F32, name=f"z{g}_{t}", tag=f"zs{g}")
                    nc.vector.tensor_tensor(
                        out=z, in0=zp,
                        in1=xwblk[:, 4 * g:4 * g + 4, toff * BL:(toff + 1) * BL],
                        op=ALU.add)
                    if g == 0:
                        s_i = loop.tile([128, HK, BL], F32, name=f"si_{t}", tag="si")
                        nc.scalar.activation(out=s_i, in_=z, func=AF.Sigmoid)
                    elif g == 1:
                        s_f = loop.tile([128, HK, BL], F32, name=f"sf_{t}", tag="sf")
                        nc.scalar.activation(out=s_f, in_=z, func=AF.Sigmoid)
                        m1 = loop.tile([128, HK, BL], F32, name=f"m1_{t}", tag="m1")
                        nc.vector.tensor_mul(m1, s_f, c_run)
                    elif g == 2:
                        t_g = loop.tile([128, HK, BL], F32, name=f"tg_{t}", tag="tg")
                        nc.scalar.activation(out=t_g, in_=z, func=AF.Tanh)
                        m2 = loop.tile([128, HK, BL], F32, name=f"m2_{t}", tag="m2")
                        nc.vector.tensor_mul(m2, s_i, t_g)
                        c_new = loop.tile([128, HK, BL], F32, name=f"cn_{t}", tag="cn")
                        nc.vector.tensor_add(c_new, m1, m2)
                        nc.vector.copy_predicated(
                            c_run.rearrange("p k b -> p (k b)"), m3,
                            c_new.rearrange("p k b -> p (k b)"))
                        tan_c = loop.tile([128, HK, BL], F32, name=f"tc_{t}", tag="tc")
                        nc.scalar.activation(out=tan_c, in_=c_run, func=AF.Tanh)
                    else:
                        s_o = loop.tile([128, HK, BL], F32, name=f"so_{t}", tag="so")
                        nc.scalar.activation(out=s_o, in_=z, func=AF.Sigmoid)
                        h_new = loop.tile([128, HK, BL], F32, name=f"hn_{t}", tag="hn")
                        nc.vector.tensor_mul(h_new, s_o, tan_c)
                        nc.vector.copy_predicated(
                            h_run.rearrange("p k b -> p (k b)"), m3,
                            h_new.rearrange("p k b -> p (k b)"))
                        nc.scalar.copy(h_bf, h_run)
                        nc.scalar.copy(hsT[:, :, t, :], h_run)

            nc.sync.dma_start(out=hc_d[0], in_=h_run)
            nc.sync.dma_start(out=hc_d[1], in_=c_run)

            # ---------- phase 3: attention ----------
            for b in range(BL):
                encT_sb = att.tile([128, HK, TE], F32, name=f"eT_{b}", tag="encT")
                nc.sync.dma_start(out=encT_sb,
                                  in_=encT_d[b].rearrange("(k p) e -> p k e", p=128))
                encT_r = attr.tile([128, HK, TE], F32R, name=f"eTr_{b}", tag="encTr")
                nc.vector.tensor_copy(encT_r, encT_sb)
                enc_sb = att.tile([128, KT, H], F32, name=f"e_{b}", tag="enc")
                nc.sync.dma_start(out=enc_sb,
                                  in_=enc_d[b].rearrange("(k p) h -> p k h", p=128))
                enc_r = attr.tile([128, KT, H], F32R, name=f"er_{b}", tag="encr")
                nc.scalar.copy(enc_r, enc_sb)

                for qt in range(QT):
                    ps_s = psA.tile([128, TE], F32, name=f"ps_{b}_{qt}", tag="mmA")
                    for k in range(HK):
                        nc.tensor.matmul(
                            out=ps_s,
                            lhsT=hsT[:, k, qt * 128:(qt + 1) * 128, b],
                            rhs=encT_r[:, k, :],
                            start=(k == 0), stop=(k == HK - 1))
                    mx = attc.tile([128, 1], F32, name=f"mx_{b}_{qt}", tag="mx")
                    nc.vector.tensor_reduce(out=mx, in_=ps_s,
                                            axis=mybir.AxisListType.X, op=ALU.max)
                    nmx = attc.tile([128, 1], F32, name=f"nmx_{b}_{qt}", tag="nmx")
                    nc.vector.tensor_scalar(out=nmx, in0=mx, scalar1=-1.0,
                                            scalar2=None, op0=ALU.mult)
                    expv = attc.tile([128, TE], F32, name=f"ex_{b}_{qt}", tag="ex")
                    se = attc.tile([128, 1], F32, name=f"se_{b}_{qt}", tag="se")
                    nc.scalar.activation(out=expv, in_=ps_s, func=AF.Exp,
                                         bias=nmx, scale=1.0, accum_out=se)
                    rse = attc.tile([128, 1], F32, name=f"rs_{b}_{qt}", tag="rs")
                    nc.vector.reciprocal(rse, se)
                    probs = attc.tile([128, TE], F32R, name=f"pr_{b}_{qt}", tag="pr")
                    nc.vector.tensor_scalar(out=probs, in0=expv, scalar1=rse,
                                            scalar2=None, op0=ALU.mult)
                    probsT = attc.tile([128, KT, 128], F32R,
                                       name=f"pT_{b}_{qt}", tag="pT")
                    for kt in range(KT):
                        ptp = psA.tile([128, 128], F32R, name=f"ptp_{b}_{qt}_{kt}",
                                       tag="mmA")
                        nc.tensor.transpose(out=ptp,
                                            in_=probs[:, kt * 128:(kt + 1) * 128],
                                            identity=ident_r)
                        nc.vector.tensor_copy(probsT[:, kt, :], ptp)
                    ps_c = psA.tile([128, H], F32, name=f"pc_{b}_{qt}", tag="mmA")
                    for kt in range(KT):
                        nc.tensor.matmul(out=ps_c, lhsT=probsT[:, kt, :],
                                         rhs=enc_r[:, kt, :],
                                         start=(kt == 0), stop=(kt == KT - 1))
                    ctx_sb = attc.tile([128, H], F32, name=f"cx_{b}_{qt}", tag="cx")
                    nc.vector.tensor_scalar(out=ctx_sb, in0=ps_c,
                                            scalar1=maskTq[:, qt, b:b + 1],
                                            scalar2=None, op0=ALU.mult)
                    nc.sync.dma_start(out=ctx_d[b, qt], in_=ctx_sb)

    nc.compile()
    return nc


# ---------------------------------------------------------------------------
# host side
# ---------------------------------------------------------------------------
_BUILT = {}
LAST_RESULTS = None  # BassKernelResults of the most recent run (for profiling)


def _get_built(T):
    if T not in _BUILT:
        _BUILT[T] = build_kernel(T)
    return _BUILT[T]


def make_in_maps(enc_outputs, dec_inputs, state_h, state_c, E, W, U, b, T=256):
    bf = ml_dtypes.bfloat16
    Eb = np.ascontiguousarray(E.astype(bf))
    Wb = np.ascontiguousarray(W.astype(bf))
    Ub = np.ascontiguousarray(U.astype(bf))
    bT = np.ascontiguousarray(b.astype(np.float32).reshape(MT, 128).T)
    n_chunks = T * BL // 128
    in_maps = []
    for c in range(NCORES):
        sl = slice(c * BL, (c + 1) * BL)
        decT = np.ascontiguousarray(dec_inputs[sl, :T].T.astype(np.int32))
        decT4 = np.ascontiguousarray(
            np.broadcast_to(decT[:, None, :], (T, HK, BL)).astype(np.int32))
        idxw = np.ascontiguousarray(decT.reshape(n_chunks, 128).T)
        h0T = np.ascontiguousarray(
            state_h[sl].astype(np.float32).T.reshape(HK, 128, BL).transpose(1, 0, 2))
        c0T = np.ascontiguousarray(
            state_c[sl].astype(np.float32).T.reshape(HK, 128, BL).transpose(1, 0, 2))
        encT = np.ascontiguousarray(
            enc_outputs[sl].astype(np.float32).transpose(0, 2, 1))
        enc = np.ascontiguousarray(enc_outputs[sl].astype(np.float32))
        in_maps.append({
            "E": Eb, "W": Wb, "U": Ub, "bT": bT, "decT": decT, "decT4": decT4,
            "idxw": idxw,
            "h0T": h0T, "c0T": c0T, "encT": encT, "enc": enc,
        })
    return in_maps


def assemble(results, T=256):
    QT = T // 128
    ctx = np.empty((B, T, H), np.float32)
    hT = np.empty((B, H), np.float32)
    cT = np.empty((B, H), np.float32)
    for c in range(NCORES):
        sl = slice(c * BL, (c + 1) * BL)
        r = results[c]
        ctx[sl] = r["ctx8"].reshape(BL, QT * 128, H)
        hc = r["hc"]  # [2, 128, HK, BL]
        hT[sl] = hc[0].transpose(2, 1, 0).reshape(BL, H)
        cT[sl] = hc[1].transpose(2, 1, 0).reshape(BL, H)
    return ctx, hT, cT


def kernel(enc_outputs, dec_inputs, state_h, state_c, E, W, U, b):
    enc_outputs = np.asarray(enc_outputs)
    dec_inputs = np.asarray(dec_inputs)
    state_h = np.asarray(state_h)
    state_c = np.asarray(state_c)
    E = np.asarray(E)
    W = np.asarray(W)
    U = np.asarray(U)
    b = np.asarray(b)
    T = dec_inputs.shape[1]
    nc = _get_built(T)
    in_maps = make_in_maps(enc_outputs, dec_inputs, state_h, state_c,
                           E, W, U, b, T=T)
    res = run_bass_kernel_spmd(nc, in_maps, core_ids=list(range(NCORES)))
    global LAST_RESULTS
    LAST_RESULTS = res
    return assemble(res.results, T=T)


# revision 23
# speedup vs baseline: 1.0861x; 1.0861x over previous
"""nn_Decoder kernel for 8 TRN2 NeuronCores.

Model: masked LSTM decoder (Keras semantics) + Luong dot attention.
  mask = dec_inputs != 0
  x = E[dec_inputs]                       [B,T,D]
  dec_outputs, hT, cT = masked_lstm(x, mask, h0, c0, W, U, b)
  scores = dec_outputs @ enc^T ; probs = softmax(scores)
  ctx = probs @ enc ; ctx[~mask] = 0
  returns (ctx, hT, cT)

Sharding: data-parallel over batch, 8 rows per core. The LSTM recurrence
runs in transposed layout (hidden dim on partitions, batch on free dim)
with U as the stationary matmul operand (bf16, FWL weight loads).
x@W is precomputed for all timesteps in one batched matmul (bf16) and
staged through DRAM in 64-step blocks. Attention runs in float32r.

Numerics (validated by emulation + a HW probe): bf16 recurrence matmuls
with fp32 cell/hidden carry give ~2.3e-3 absmax error vs fp32 reference;
float32r matmul measures 1.5e-4 relative error on HW.
"""
import numpy as np
import ml_dtypes
from contextlib import ExitStack

import concourse.bass as bass
import concourse.bacc as bacc
import concourse.tile as tile
from concourse import mybir
from concourse.bass_utils import run_bass_kernel_spmd
from concourse.masks import make_identity

AF = mybir.ActivationFunctionType
ALU = mybir.AluOpType
DT = mybir.dt
F32 = DT.float32
F32R = DT.float32r
BF16 = DT.bfloat16
I32 = DT.int32

B, TE, H, D, V = 64, 256, 512, 256, 50000
FH = 4 * H            # 2048
NCORES = 8
BL = B // NCORES      # 8 batch rows per core
HK = H // 128         # 4 contraction k-tiles over H
MT = FH // 128        # 16 m-tiles over 4H
DK = D // 128         # 2 k-tiles over D
KT = TE // 128        # 2 key tiles for attention
XWBLK = 32            # timesteps per XW^T DRAM->SBUF block


def build_kernel(T=256):
    NF = T * BL                      # flats = (t, b) pairs, t-major
    n_chunks = NF // 128             # gather chunks of 128 rows
    QT = T // 128                    # query tiles for attention

    nc = bacc.Bacc("TRN2", target_bir_lowering=False, debug=False,
                   num_devices=NCORES)

    E_d = nc.dram_tensor("E", [V, D], BF16, kind="ExternalInput").ap()
    W_d = nc.dram_tensor("W", [D, FH], BF16, kind="ExternalInput").ap()
    U_d = nc.dram_tensor("U", [H, FH], BF16, kind="ExternalInput").ap()
    bT_d = nc.dram_tensor("bT", [128, MT], F32, kind="ExternalInput").ap()
    decT_d = nc.dram_tensor("decT", [T, BL], I32, kind="ExternalInput").ap()
    decT4_d = nc.dram_tensor("decT4", [T, HK, BL], I32, kind="ExternalInput").ap()
    idxw_d = nc.dram_tensor("idxw", [128, n_chunks], I32, kind="ExternalInput").ap()
    h0_d = nc.dram_tensor("h0T", [128, HK, BL], F32, kind="ExternalInput").ap()
    c0_d = nc.dram_tensor("c0T", [128, HK, BL], F32, kind="ExternalInput").ap()
    encT_d = nc.dram_tensor("encT", [BL, H, TE], F32, kind="ExternalInput").ap()
    enc_d = nc.dram_tensor("enc", [BL, TE, H], F32, kind="ExternalInput").ap()

    ctx_d = nc.dram_tensor("ctx8", [BL, QT, 128, H], F32, kind="ExternalOutput").ap()
    hc_d = nc.dram_tensor("hc", [2, 128, HK, BL], F32, kind="ExternalOutput").ap()

    with tile.TileContext(nc) as tc:
        with ExitStack() as ctx:
            sing = ctx.enter_context(tc.tile_pool(name="sing", bufs=1))
            ph1 = ctx.enter_context(tc.tile_pool(name="ph1", bufs=1))
            xpool = ctx.enter_context(tc.tile_pool(name="xpool", bufs=3))
            xwst = ctx.enter_context(tc.tile_pool(name="xwst", bufs=3))
            xwbp = ctx.enter_context(tc.tile_pool(name="xwbp", bufs=2))
            loop = ctx.enter_context(tc.tile_pool(name="loop", bufs=3))
            att = ctx.enter_context(tc.tile_pool(name="att", bufs=2))
            attr = ctx.enter_context(tc.tile_pool(name="attr", bufs=1))
            attc = ctx.enter_context(tc.tile_pool(name="attc", bufs=2))
            mseg = ctx.enter_context(tc.tile_pool(name="mseg", bufs=2))
            dram = ctx.enter_context(tc.tile_pool(name="dram", bufs=1, space="DRAM"))
            psA = ctx.enter_context(tc.tile_pool(name="psA", bufs=3, space="PSUM"))
            psZ = ctx.enter_context(tc.tile_pool(name="psZ", bufs=2, space="PSUM"))

            # ---------- setup ----------
            ident_bf = sing.tile([128, 128], BF16)
            make_identity(nc, ident_bf)
            ident_f = sing.tile([128, 128], F32)
            make_identity(nc, ident_f)
            ident_r = sing.tile([128, 128], F32R)
            nc.vector.tensor_copy(ident_r, ident_f)

            U_bf = sing.tile([128, HK, FH], BF16)
            nc.sync.dma_start(out=U_bf, in_=U_d.rearrange("(k p) f -> p k f", p=128))
            W_bf = ph1.tile([128, DK, FH], BF16)
            nc.sync.dma_start(out=W_bf, in_=W_d.rearrange("(k p) f -> p k f", p=128))
            bT_sb = sing.tile([128, MT], F32)
            nc.sync.dma_start(out=bT_sb, in_=bT_d)
            idx_sb = sing.tile([128, n_chunks], I32)
            nc.sync.dma_start(out=idx_sb, in_=idxw_d)
            h_run = sing.tile([128, HK, BL], F32)
            nc.sync.dma_start(out=h_run, in_=h0_d)
            c_run = sing.tile([128, HK, BL], F32)
            nc.sync.dma_start(out=c_run, in_=c0_d)
            h_bf = sing.tile([128, HK, BL], BF16)
            nc.scalar.copy(h_bf, h_run)

            # masks
            decq_sb = sing.tile([128, QT, BL], I32)
            nc.sync.dma_start(out=decq_sb,
                              in_=decT_d.rearrange("(q p) b -> p q b", p=128))
            maskTq = sing.tile([128, QT, BL], F32)
            nc.vector.tensor_scalar(out=maskTq, in0=decq_sb, scalar1=0,
                                    scalar2=None, op0=ALU.not_equal)
            # mask broadcast to all 128 partitions via stride-0 DMA reads
            maskbc = sing.tile([128, NF * HK], DT.uint8)
            dec4_flat = decT4_d.rearrange("t k b -> (t k b)")
            seg = NF * HK // 4
            for s in range(4):
                mstg = mseg.tile([128, seg], I32, name=f"mstg_{s}", tag="mstg")
                src = bass.AP(tensor=dec4_flat.tensor,
                              offset=dec4_flat.offset + s * seg,
                              ap=[[0, 128], [1, seg]])
                nc.sync.dma_start(out=mstg, in_=src)
                nc.vector.tensor_scalar(out=maskbc[:, s * seg:(s + 1) * seg],
                                        in0=mstg, scalar1=0,
                                        scalar2=None, op0=ALU.not_equal)

            hsT = sing.tile([128, HK, T, BL], F32)
            xw_dram = dram.tile([128, MT, NF], BF16)

            # ---------- phase 1: embedding gather + XW^T ----------
            # xw_dram m-positions are gate-major remapped so the recurrence
            # banks are contiguous: bank B = [g, i] at 0..7, bank A = [f, o]
            # at 8..15 (original gate order in z is i,f,g,o).
            REMAP = {0: 4, 1: 8, 2: 0, 3: 12}  # orig gate -> dram base pos
            xT = ph1.tile([128, DK, NF], BF16)
            n_nc = NF // 512
            for c in range(n_nc):
                for rr in range(4):
                    r = c * 4 + rr
                    x_sb = xpool.tile([128, D], BF16, name=f"x_{r}", tag="x")
                    nc.gpsimd.indirect_dma_start(
                        out=x_sb, out_offset=None, in_=E_d,
                        in_offset=bass.IndirectOffsetOnAxis(
                            ap=idx_sb[:, r:r + 1], axis=0))
                    for k in range(DK):
                        pt = psA.tile([128, 128], BF16, name=f"pt_{r}_{k}",
                                      tag="mmA")
                        nc.tensor.transpose(
                            out=pt, in_=x_sb[:, k * 128:(k + 1) * 128],
                            identity=ident_bf)
                        nc.vector.tensor_copy(xT[:, k, r * 128:(r + 1) * 128], pt)
                for m in range(MT):
                    g_orig, j = divmod(m, HK)
                    mpos = REMAP[g_orig] + j
                    pxw = psA.tile([128, 512], F32, name=f"pxw_{m}_{c}", tag="mmA")
                    for k in range(DK):
                        nc.tensor.matmul(out=pxw,
                                         lhsT=W_bf[:, k, m * 128:(m + 1) * 128],
                                         rhs=xT[:, k, c * 512:(c + 1) * 512],
                                         start=(k == 0), stop=(k == DK - 1))
                    stg = xwst.tile([128, 512], BF16, name=f"stg_{m}_{c}", tag="stg")
                    if (m + c) % 2 == 0:
                        nc.scalar.activation(out=stg, in_=pxw, func=AF.Identity,
                                             bias=bT_sb[:, m:m + 1], scale=1.0)
                    else:
                        nc.vector.tensor_scalar(out=stg, in0=pxw,
                                                scalar1=bT_sb[:, m:m + 1],
                                                scalar2=None, op0=ALU.add)
                    nc.sync.dma_start(out=xw_dram[:, mpos, c * 512:(c + 1) * 512],
                                      in_=stg)

            # ---------- phase 2: recurrence ----------
            xwblk = None
            for t in range(T):
                blk, toff = divmod(t, XWBLK)
                if toff == 0:
                    nb = min(XWBLK, T - blk * XWBLK)
                    xwblk = xwbp.tile([128, MT, XWBLK * BL], BF16,
                                      name=f"xwblk_{blk}", tag="xwblk")
                    nc.sync.dma_start(
                        out=xwblk[:, :, :nb * BL],
                        in_=xw_dram[:, :, blk * XWBLK * BL:(blk * XWBLK + nb) * BL])

                KB = HK * BL
                m3 = maskbc[:, t * KB:(t + 1) * KB]
                xo = toff * BL

                # xw pre-seed into PSUM via identity matmuls (start=True);
                # these depend only on xw, so they run in the PE idle gap
                # while the previous step's tail computes.
                zB = psZ.tile([128, 2 * HK, BL], F32, name=f"zB_{t}", tag="zB")
                zA = psZ.tile([128, 2 * HK, BL], F32, name=f"zA_{t}", tag="zA")
                nc.tensor.matmul(out=zB, lhsT=ident_bf,
                                 rhs=xwblk[:, 0:8, xo:xo + BL],
                                 start=True, stop=False, skip_group_check=True)
                nc.tensor.matmul(out=zA, lhsT=ident_bf,
                                 rhs=xwblk[:, 8:16, xo:xo + BL],
                                 start=True, stop=False, skip_group_check=True)

                # bank B: slots 0-3 = gate g (tanh), 4-7 = gate i
                for slot, g_orig in ((0, 2), (1, 0)):
                    for j in range(HK):
                        mtile = g_orig * HK + j
                        for k in range(HK):
                            nc.tensor.matmul(
                                out=zB[:, slot * HK + j, :],
                                lhsT=U_bf[:, k, mtile * 128:(mtile + 1) * 128],
                                rhs=h_bf[:, k, :],
                                start=False, stop=(k == HK - 1),
                                skip_group_check=True)
                t_g = loop.tile([128, HK, BL], F32, name=f"tg_{t}", tag="tg")
                nc.scalar.activation(out=t_g, in_=zB[:, 0:HK, :], func=AF.Tanh)
                s_i = loop.tile([128, HK, BL], F32, name=f"si_{t}", tag="si")
                nc.scalar.activation(out=s_i, in_=zB[:, HK:2 * HK, :],
                                     func=AF.Sigmoid)
                m2 = loop.tile([128, HK, BL], F32, name=f"m2_{t}", tag="m2")
                nc.vector.tensor_mul(m2, s_i, t_g)

                # bank A: slots 0-3 = gate f, 4-7 = gate o
                for slot, g_orig in ((0, 1), (1, 3)):
                    for j in range(HK):
                        mtile = g_orig * HK + j
                        for k in range(HK):
                            nc.tensor.matmul(
                                out=zA[:, slot * HK + j, :],
                                lhsT=U_bf[:, k, mtile * 128:(mtile + 1) * 128],
                                rhs=h_bf[:, k, :],
                                start=False, stop=(k == HK - 1),
                                skip_group_check=True)
                s_fo = loop.tile([128, 2 * HK, BL], F32, name=f"sfo_{t}", tag="sfo")
                nc.scalar.activation(out=s_fo, in_=zA, func=AF.Sigmoid)
                m1 = loop.tile([128, HK, BL], F32, name=f"m1_{t}", tag="m1")
                nc.vector.tensor_mul(m1, s_fo[:, 0:HK, :], c_run)
                c_new = loop.tile([128, HK, BL], F32, name=f"cn_{t}", tag="cn")
                nc.vector.tensor_add(c_new, m1, m2)
                # tanh(c_new) unmasked: masked lanes are discarded by the
                # predicated copies below, so we don't wait for c_run update
                tan_c = loop.tile([128, HK, BL], F32, name=f"tc_{t}", tag="tc")
                nc.scalar.activation(out=tan_c, in_=c_new, func=AF.Tanh)
                hn_bf = loop.tile([128, HK, BL], BF16, name=f"hb_{t}", tag="hb")
                nc.vector.tensor_mul(hn_bf, s_fo[:, HK:2 * HK, :], tan_c)
                nc.vector.copy_predicated(
                    h_bf.rearrange("p k b -> p (k b)"), m3,
                    hn_bf.rearrange("p k b -> p (k b)"))
                # f32 shadow state (outputs + attention), off the critical path
                h_new = loop.tile([128, HK, BL], F32, name=f"hn_{t}", tag="hn")
                nc.vector.tensor_mul(h_new, s_fo[:, HK:2 * HK, :], tan_c)
                nc.vector.copy_predicated(
                    h_run.rearrange("p k b -> p (k b)"), m3,
                    h_new.rearrange("p k b -> p (k b)"))
                nc.vector.copy_predicated(
                    c_run.rearrange("p k b -> p (k b)"), m3,
                    c_new.rearrange("p k b -> p (k b)"))
                nc.gpsimd.tensor_copy(out=hsT[:, :, t, :], in_=h_run)

            nc.sync.dma_start(out=hc_d[0], in_=h_run)
            nc.sync.dma_start(out=hc_d[1], in_=c_run)

            # ---------- phase 3: attention ----------
            # qt-outer: all qt=0 work only needs hsT[t<128] and overlaps the
            # second half of the recurrence; qt=1 is the only post-loop tail.
            for qt in range(QT):
                for b in range(BL):
                    encT_sb = att.tile([128, HK, TE], F32,
                                       name=f"eT_{qt}_{b}", tag="encT")
                    nc.sync.dma_start(
                        out=encT_sb,
                        in_=encT_d[b].rearrange("(k p) e -> p k e", p=128))
                    encT_r = attr.tile([128, HK, TE], F32R,
                                       name=f"eTr_{qt}_{b}", tag="encTr")
                    nc.vector.tensor_copy(encT_r, encT_sb)
                    enc_sb = att.tile([128, KT, H], F32,
                                      name=f"e_{qt}_{b}", tag="enc")
                    nc.sync.dma_start(
                        out=enc_sb,
                        in_=enc_d[b].rearrange("(k p) h -> p k h", p=128))
                    enc_r = attr.tile([128, KT, H], F32R,
                                      name=f"er_{qt}_{b}", tag="encr")
                    nc.scalar.copy(enc_r, enc_sb)
                    hsr = att.tile([128, HK, 128], F32R,
                                   name=f"hsr_{qt}_{b}", tag="hsr")
                    nc.vector.tensor_copy(
                        hsr, hsT[:, :, qt * 128:(qt + 1) * 128, b])

                    ps_s = psA.tile([128, TE], F32, name=f"ps_{b}_{qt}", tag="mmA")
                    for k in range(HK):
                        nc.tensor.matmul(
                            out=ps_s,
                            lhsT=hsr[:, k, :],
                            rhs=encT_r[:, k, :],
                            start=(k == 0), stop=(k == HK - 1))
                    mx = attc.tile([128, 1], F32, name=f"mx_{b}_{qt}", tag="mx")
                    nc.vector.tensor_reduce(out=mx, in_=ps_s,
                                            axis=mybir.AxisListType.X, op=ALU.max)
                    nmx = attc.tile([128, 1], F32, name=f"nmx_{b}_{qt}", tag="nmx")
                    nc.vector.tensor_scalar(out=nmx, in0=mx, scalar1=-1.0,
                                            scalar2=None, op0=ALU.mult)
                    expv = attc.tile([128, TE], F32, name=f"ex_{b}_{qt}", tag="ex")
                    se = attc.tile([128, 1], F32, name=f"se_{b}_{qt}", tag="se")
                    nc.scalar.activation(out=expv, in_=ps_s, func=AF.Exp,
                                         bias=nmx, scale=1.0, accum_out=se)
                    rse = attc.tile([128, 1], F32, name=f"rs_{b}_{qt}", tag="rs")
                    nc.vector.reciprocal(rse, se)
                    probs = attc.tile([128, TE], F32R, name=f"pr_{b}_{qt}", tag="pr")
                    nc.vector.tensor_scalar(out=probs, in0=expv, scalar1=rse,
                                            scalar2=None, op0=ALU.mult)
                    probsT = attc.tile([128, KT, 128], F32R,
                                       name=f"pT_{b}_{qt}", tag="pT")
                    for kt in range(KT):
                        ptp = psA.tile([128, 128], F32R, name=f"ptp_{b}_{qt}_{kt}",
                                       tag="mmA")
                        nc.tensor.transpose(out=ptp,
                                            in_=probs[:, kt * 128:(kt + 1) * 128],
                                            identity=ident_r)
                        nc.vector.tensor_copy(probsT[:, kt, :], ptp)
                    ps_c = psA.tile([128, H], F32, name=f"pc_{b}_{qt}", tag="mmA")
                    for kt in range(KT):
                        nc.tensor.matmul(out=ps_c, lhsT=probsT[:, kt, :],
                                         rhs=enc_r[:, kt, :],
                                         start=(kt == 0), stop=(kt == KT - 1))
                    ctx_sb = attc.tile([128, H], F32, name=f"cx_{b}_{qt}", tag="cx")
                    nc.vector.tensor_scalar(out=ctx_sb, in0=ps_c,
                                            scalar1=maskTq[:, qt, b:b + 1],
                                            scalar2=None, op0=ALU.mult)
                    nc.sync.dma_start(out=ctx_d[b, qt], in_=ctx_sb)

    nc.compile()
    return nc


# ---------------------------------------------------------------------------
# host side
# ---------------------------------------------------------------------------
_BUILT = {}
LAST_RESULTS = None  # BassKernelResults of the most recent run (for profiling)


def _get_built(T):
    if T not in _BUILT:
        _BUILT[T] = build_kernel(T)
    return _BUILT[T]


def make_in_maps(enc_outputs, dec_inputs, state_h, state_c, E, W, U, b, T=256):
    bf = ml_dtypes.bfloat16
    Eb = np.ascontiguousarray(E.astype(bf))
    Wb = np.ascontiguousarray(W.astype(bf))
    Ub = np.ascontiguousarray(U.astype(bf))
    bT = np.ascontiguousarray(b.astype(np.float32).reshape(MT, 128).T)
    n_chunks = T * BL // 128
    in_maps = []
    for c in range(NCORES):
        sl = slice(c * BL, (c + 1) * BL)
        decT = np.ascontiguousarray(dec_inputs[sl, :T].T.astype(np.int32))
        decT4 = np.ascontiguousarray(
            np.broadcast_to(decT[:, None, :], (T, HK, BL)).astype(np.int32))
        idxw = np.ascontiguousarray(decT.reshape(n_chunks, 128).T)
        h0T = np.ascontiguousarray(
            state_h[sl].astype(np.float32).T.reshape(HK, 128, BL).transpose(1, 0, 2))
        c0T = np.ascontiguousarray(
            state_c[sl].astype(np.float32).T.reshape(HK, 128, BL).transpose(1, 0, 2))
        encT = np.ascontiguousarray(
            enc_outputs[sl].astype(np.float32).transpose(0, 2, 1))
        enc = np.ascontiguousarray(enc_outputs[sl].astype(np.float32))
        in_maps.append({
            "E": Eb, "W": Wb, "U": Ub, "bT": bT, "decT": decT, "decT4": decT4,
            "idxw": idxw,
            "h0T": h0T, "c0T": c0T, "encT": encT, "enc": enc,
        })
    return in_maps


def assemble(results, T=256):
    QT = T // 128
    ctx = np.empty((B, T, H), np.float32)
    hT = np.empty((B, H), np.float32)
    cT = np.empty((B, H), np.float32)
    for c in range(NCORES):
        sl = slice(c * BL, (c + 1) * BL)
        r = results[c]
        ctx[sl] = r["ctx8"].reshape(BL, QT * 128, H)
        hc = r["hc"]  # [2, 128, HK, BL]
        hT[sl] = hc[0].transpose(2, 1, 0).reshape(BL, H)
        cT[sl] = hc[1].transpose(2, 1, 0).reshape(BL, H)
    return ctx, hT, cT


def kernel(enc_outputs, dec_inputs, state_h, state_c, E, W, U, b):
    enc_outputs = np.asarray(enc_outputs)
    dec_inputs = np.asarray(dec_inputs)
    state_h = np.asarray(state_h)
    state_c = np.asarray(state_c)
    E = np.asarray(E)
    W = np.asarray(W)
    U = np.asarray(U)
    b = np.asarray(b)
    T = dec_inputs.shape[1]
    nc = _get_built(T)
    in_maps = make_in_maps(enc_outputs, dec_inputs, state_h, state_c,
                           E, W, U, b, T=T)
    res = run_bass_kernel_spmd(nc, in_maps, core_ids=list(range(NCORES)))
    global LAST_RESULTS
    LAST_RESULTS = res
    return assemble(res.results, T=T)
